# revision 9
# baseline (speedup 1.0000x reference)
"""DetNet Trainium2 kernel v2: 90-layer MLP recurrence, data-parallel over 8 cores.

Per core (2048 samples), features on partitions, batch on free axis.
Layout (partition sections chosen so every engine op is same-base, and
non-base-0 sections are 32-aligned and <=32 rows):
  cA   [126,2048] f32r: v(0:60) | pad | tH(64:94) | pad | Hr(96:126)
  trep [120,2048] f16 : t replicated 4x, block j4 at partitions 30*j4+k
  ttp  [94,2048]  f32 : t_tilde state at 64:94
  tt16 [94,2048]  f16 : clipped t at 64:94 (output + replicate source)

Per layer, per chunk (CH=512):
  einsum tH[b] = HH[b]^T t[b]:
     DVE : P[(j4,k),(g,b)] = trep * hhy      (all-fp16 -> 2x mode)
     PE  : 8 selector matmuls (fp16) accumulate tH in psum[64:94]
     ACT : copy psum -> cA[64:94]
  mm1  : z = W1 @ [cA; t]    (PE, 2 k-tiles x 2 m-tiles; t-tile in fp16)
  relu : ACT (bias b1), zA/zB f32r
  mm23 : tv[0:60]=W3@z, tv[64:94]=W2@z  (PE, fused M=94, 2 k-tiles)
  ACT  : vtmp = Identity(tv[0:60]+b3)   (GPSIMD cannot read PSUM)
  Pool : cA[0:60] += vtmp  (tensor_tensor add, SBUF-only)
  DVE  : ttp += tv[64:94]+b2 (scalar_tensor_tensor, PSUM read)
  Pool : u1 = max(ik*ttp, -1) f16 (SBUF-only)
  DVE  : tt16 = min(u1, 1)   (all-fp16 -> 4x mode)
  SP   : weights + 4 replicate DMAs tt16[64:94] -> trep blocks (HW DGE, async)
"""
import sys
import numpy as np

sys.path.insert(0, "/opt/trn_rl_repo")

from contextlib import ExitStack

import concourse.bass as bass
import concourse.tile as tile
from concourse import mybir
from concourse.bass_utils import run_bass_kernel_spmd

B = 16384
K = 30
LAYERS = 90
VL = 60
ZL = 240
NCORES = 8
BC = B // NCORES          # 2048
NCHUNK = 4
CH = BC // NCHUNK         # 512

F32 = mybir.dt.float32
MM_DT = mybir.dt.float32r   # fp32 data on the fast PE path (1 cyc/row at N>=256)
F16 = mybir.dt.float16

AO = mybir.AluOpType
RELU = mybir.ActivationFunctionType.Relu
IDENT = mybir.ActivationFunctionType.Identity
LAST_RESULT = None  # BassKernelResults of the most recent run (for profiling)


def build_kernel(inv_kap):
    nc = bass.Bass()

    hr_in = nc.declare_dram_parameter("HrT", [K, BC], MM_DT, isOutput=False)
    hhy_in = nc.declare_dram_parameter("HHY", [120, 8 * BC], F16, isOutput=False)
    w1a_in = nc.declare_dram_parameter("W1A", [LAYERS, 126, ZL], MM_DT, isOutput=False)
    w1b_in = nc.declare_dram_parameter("W1B", [LAYERS, K, ZL], F16, isOutput=False)
    w23a_in = nc.declare_dram_parameter("W23A", [LAYERS, 128, 94], MM_DT, isOutput=False)
    w23b_in = nc.declare_dram_parameter("W23B", [LAYERS, 112, 94], MM_DT, isOutput=False)
    bias_in = nc.declare_dram_parameter("BIAS", [LAYERS, 128, 4], F32, isOutput=False)
    sel_in = nc.declare_dram_parameter("SEL", [120, 8 * K], F16, isOutput=False)
    zero32_in = nc.declare_dram_parameter("ZERO32", [126, BC], MM_DT, isOutput=False)
    zero16_in = nc.declare_dram_parameter("ZERO16", [120, BC], F16, isOutput=False)
    out_dram = nc.declare_dram_parameter("OUT", [LAYERS, K, BC], F16, isOutput=True)

    with tile.TileContext(nc) as tc, ExitStack() as ctx:
        persist = ctx.enter_context(tc.tile_pool(name="persist", bufs=1))
        wpool = ctx.enter_context(tc.tile_pool(name="w", bufs=3))
        zpool = ctx.enter_context(tc.tile_pool(name="z", bufs=3))
        upool = ctx.enter_context(tc.tile_pool(name="u", bufs=3))
        pp_z = ctx.enter_context(tc.tile_pool(name="ps_z", bufs=2, space="PSUM"))
        pp_t = ctx.enter_context(tc.tile_pool(name="ps_t", bufs=2, space="PSUM"))
        pp_h = ctx.enter_context(tc.tile_pool(name="ps_h", bufs=2, space="PSUM"))

        # ---- persistent state
        cA = persist.tile([126, BC], MM_DT)     # v | tH | Hr
        trep = persist.tile([120, BC], F16)     # t rep4 (einsum + mm1 rhs)
        ttp = persist.tile([94, BC], F32)       # t_tilde at 64:94
        tt16 = persist.tile([94, BC], F16)      # clipped t at 64:94
        hhy = persist.tile([120, 8, BC], F16)   # HH rearranged
        pbuf = persist.tile([120, 8, BC], F16)  # einsum products
        sel = persist.tile([120, 8, K], F16)    # selector weights

        nc.gpsimd.dma_start(cA[:], zero32_in[:])
        nc.gpsimd.dma_start(trep[:], zero16_in[:])
        nc.vector.memset(ttp[:], 0.0)
        nc.gpsimd.dma_start(cA[96 : 96 + K, :], hr_in[:])
        nc.gpsimd.dma_start(hhy[:].rearrange("p a b -> p (a b)"), hhy_in[:])
        nc.gpsimd.dma_start(sel[:].rearrange("p a b -> p (a b)"), sel_in[:])

        for l in range(LAYERS):
            w1a = wpool.tile([126, ZL], MM_DT, tag="w1a")
            w1b = wpool.tile([K, ZL], F16, tag="w1b")
            w23a = wpool.tile([128, 94], MM_DT, tag="w23a")
            w23b = wpool.tile([112, 94], MM_DT, tag="w23b")
            bt = wpool.tile([128, 4], F32, tag="bias")
            nc.sync.dma_start(w1a[:], w1a_in[l])
            nc.sync.dma_start(w1b[:], w1b_in[l])
            nc.sync.dma_start(w23a[:], w23a_in[l])
            nc.sync.dma_start(w23b[:], w23b_in[l])
            nc.sync.dma_start(bt[:], bias_in[l])
            b1a = bt[0:128, 0:1]
            b1b = bt[0:112, 1:2]
            b2 = bt[64:94, 2:3]
            b3 = bt[0:VL, 3:4]

            ik = float(inv_kap[l])

            for c in range(NCHUNK):
                cs = bass.ts(c, CH)
                # ---- einsum producing tH for THIS layer (needs t of prev layer)
                if l > 0:
                    nc.vector.tensor_tensor(
                        pbuf[:, :, cs],
                        trep[:, cs].unsqueeze(1).broadcast_to((120, 8, CH)),
                        hhy[:, :, cs],
                        op=AO.mult,
                    )
                    thp = pp_h.tile([128, CH], F32, tag="th")
                    for g in range(8):
                        nc.tensor.matmul(
                            thp[64 : 64 + K, :],
                            sel[:, g, :],
                            pbuf[:, g, cs],
                            start=(g == 0),
                            stop=(g == 7),
                            tile_position=(0, 64),
                        )
                    nc.scalar.copy(cA[64 : 64 + K, cs], thp[64 : 64 + K, :])

                # ---- mm1: z pre-activation
                zp1 = pp_z.tile([128, CH], F32, tag="z1")
                zp2 = pp_z.tile([112, CH], F32, tag="z2")
                rA = cA[:, cs]
                rB = trep[0:K, cs]
                nc.tensor.matmul(zp1[:], w1a[:, 0:128], rA, start=True, stop=False)
                nc.tensor.matmul(zp1[:], w1b[:, 0:128], rB, start=False, stop=True)
                nc.tensor.matmul(zp2[:], w1a[:, 128:240], rA, start=True, stop=False)
                nc.tensor.matmul(zp2[:], w1b[:, 128:240], rB, start=False, stop=True)

                # ---- z = relu(zpre + b1) on ACT
                zA = zpool.tile([128, CH], MM_DT, tag="zA")
                zB = zpool.tile([112, CH], MM_DT, tag="zB")
                nc.scalar.activation(zA[:], zp1[:], RELU, bias=b1a)
                nc.scalar.activation(zB[:], zp2[:], RELU, bias=b1b)

                # ---- mm2+mm3 fused: tv[0:60]=dv, tv[64:94]=dt
                tv = pp_t.tile([94, CH], F32, tag="tv")
                nc.tensor.matmul(tv[:], w23a[:], zA[:], start=True, stop=False)
                nc.tensor.matmul(tv[:], w23b[:], zB[:], start=False, stop=True)

                # ---- state updates (+ biases); GPSIMD can't read PSUM
                vtmp = zpool.tile([VL, CH], MM_DT, tag="vtmp")
                nc.scalar.activation(vtmp[:], tv[0:VL, :], IDENT, bias=b3)
                nc.gpsimd.tensor_tensor(
                    cA[0:VL, cs], vtmp[:], cA[0:VL, cs], op=AO.add)
                nc.vector.scalar_tensor_tensor(
                    ttp[64:94, cs], tv[64:94, :], b2, ttp[64:94, cs],
                    op0=AO.add, op1=AO.add)

                # ---- hardtanh: t = clip(ttilde * ik, -1, 1)
                u1 = upool.tile([94, CH], F16, tag="u1")
                nc.gpsimd.tensor_scalar(
                    u1[64:94, :], ttp[64:94, cs], ik, -1.0, op0=AO.mult, op1=AO.max)
                nc.vector.tensor_scalar(
                    tt16[64:94, cs], u1[64:94, :], 1.0, None, op0=AO.min)

                # ---- replicate t into the 4 einsum blocks (async HW-DGE)
                for r in range(4):
                    nc.sync.dma_start(trep[30 * r : 30 * r + K, cs], tt16[64:94, cs])

            # ---- emit t of this layer
            nc.sync.dma_start(out_dram[l], tt16[64:94, :])

    _split_waits(nc)
    return nc


def _split_waits(nc, limit=1):
    """This toolchain build only accepts one sem-wait per instruction;
    hoist surplus waits onto same-engine NoOps inserted before the inst."""
    ctr = 0
    for f in nc.m.functions:
        for blk in f.blocks:
            insts = blk.instructions
            if not any(
                i.sync_info and i.sync_info.on_wait and len(i.sync_info.on_wait) > limit
                for i in insts
            ):
                continue
            new = []
            for inst in insts:
                si = inst.sync_info
                if si and si.on_wait and len(si.on_wait) > limit:
                    waits = list(si.on_wait)
                    extra, keep = waits[:-limit], waits[-limit:]
                    for w in extra:
                        ctr += 1
                        n = mybir.InstNoOp(name=f"WSPLIT-{ctr}", ins=[], outs=[])
                        n.engine = inst.engine
                        n.sync_info = mybir.SyncInfo(on_wait=[w], on_update=[])
                        new.append(n)
                    si.on_wait = keep
                new.append(inst)
            blk.instructions = new
    return ctr


def _prep_shared(W1, b1, W2, b2, W3, b3):
    L = W1.shape[0]
    # cA row order: [v(0:60) | pad4 | tH(64:94) | pad2 | Hr(96:126)]
    W1A = np.zeros((L, 126, ZL), np.float32)
    W1A[:, 0:VL] = W1[:, :, 30:90].transpose(0, 2, 1)        # v cols
    W1A[:, 64:94] = W1[:, :, 120:150].transpose(0, 2, 1)     # tH cols
    W1A[:, 96:126] = W1[:, :, 0:30].transpose(0, 2, 1)       # Hr cols
    W1B = np.ascontiguousarray(
        W1[:, :, 90:120].transpose(0, 2, 1)).astype(np.float16)  # t cols [L,30,240]

    # fused mm2+mm3 weight: out rows v at 0:60, t_tilde at 64:94
    W23 = np.zeros((L, ZL, 94), np.float32)
    W23[:, :, 0:VL] = W3.transpose(0, 2, 1)
    W23[:, :, 64:94] = W2.transpose(0, 2, 1)
    W23A = np.ascontiguousarray(W23[:, 0:128])
    W23B = np.ascontiguousarray(W23[:, 128:240])

    BIAS = np.zeros((L, 128, 4), np.float32)
    BIAS[:, 0:128, 0] = b1[:, 0:128]
    BIAS[:, 0:112, 1] = b1[:, 128:240]
    BIAS[:, 64:94, 2] = b2
    BIAS[:, 0:VL, 3] = b3

    # selector: thp[j] = sum_k t[k]*HH[k,j];  p = j4*30+k, group g = j//4
    SEL = np.zeros((120, 8, K), np.float16)
    for j in range(K):
        g, j4 = j // 4, j % 4
        SEL[j4 * K : (j4 + 1) * K, g, j] = 1.0
    return W1A, W1B, W23A, W23B, BIAS, SEL.reshape(120, 8 * K)


def kernel(Hr, HH, W1, b1, W2, b2, W3, b3, kappa):
    Hr = np.asarray(Hr, np.float32)
    HH = np.asarray(HH, np.float32)
    W1 = np.asarray(W1, np.float32)
    b1 = np.asarray(b1, np.float32)
    W2 = np.asarray(W2, np.float32)
    b2 = np.asarray(b2, np.float32)
    W3 = np.asarray(W3, np.float32)
    b3 = np.asarray(b3, np.float32)
    kappa = np.asarray(kappa, np.float32)

    W1A, W1B, W23A, W23B, BIAS, SELh = _prep_shared(W1, b1, W2, b2, W3, b3)
    inv_kap = (1.0 / np.abs(kappa)).astype(np.float32)

    in_maps = []
    for ci in range(NCORES):
        sl = slice(ci * BC, (ci + 1) * BC)
        HrT = np.ascontiguousarray(Hr[sl].T)
        HHp = np.zeros((BC, K, 32), np.float32)
        HHp[:, :, :K] = HH[sl]
        # HHY[p=(j4*30+k), (g, b)] = HH[b, k, g*4+j4]
        HHY = HHp.reshape(BC, K, 8, 4).transpose(3, 1, 2, 0).reshape(120, 8 * BC)
        in_maps.append({
            "HrT": HrT, "HHY": HHY.astype(np.float16),
            "W1A": W1A, "W1B": W1B, "W23A": W23A, "W23B": W23B,
            "BIAS": BIAS, "SEL": SELh,
            "ZERO32": np.zeros((126, BC), np.float32),
            "ZERO16": np.zeros((120, BC), np.float16),
        })

    nc = build_kernel(inv_kap)
    res = run_bass_kernel_spmd(nc, in_maps, list(range(NCORES)))
    global LAST_RESULT
    LAST_RESULT = res
    out = np.concatenate(
        [r["OUT"].transpose(0, 2, 1) for r in res.results], axis=1
    )
    return np.ascontiguousarray(out.astype(np.float32))


# revision 11
# speedup vs baseline: 1.9699x; 1.9699x over previous
"""DetNet Trainium2 kernel v6: 90-layer MLP recurrence, data-parallel over 8 cores.

Per core (2048 samples), features on partitions, batch on free axis.
Partition layouts keep every engine op same-base (non-0 bases 32-aligned,
<=32 rows):
  cA   [126,2048] f32r: v(0:60) | pad | tH(64:94) | pad | Hr(96:126)
  trep [120,2048] f16 : t replicated 4x, block j4 at partitions 30*j4+k
  tr_state (PSUM, [120,512] x 4 chunks): t_tilde accumulated IN PSUM across
       all 90 layers by the W2-rep4 matmul (start only at l=0); b2 enters
       via a ones-row appended to zB. ik-scaling applied at read by ACT.

Per layer, per chunk (CH=512):
  DVE : P[(j4,k),(g,b)] = trep * hhy        (all-fp16, 2x mode)
  PE  : 8 selector matmuls (fp16) -> thp psum[64:94]
  ACT : copy thp -> cA[64:94]
  PE  : mm1 z = W1 @ [cA; t] (2 k-tiles x 2 m-tiles, t-tile fp16)
  ACT : zA=relu(zp1+b1a), zB=relu(zp2+b1b)  (zB has ones row 112 for b2)
  PE  : mm3 vp = W3 @ z ; mm2r tr_state += W2rep4' @ [zA;zB1]  (accumulate)
  ACT : vtmp = Identity(vp + b3)
  Pool: cA[0:60] += vtmp  (gpsimd DMA accum_op=add, SBUF->SBUF)
  ACT : u = Identity(ik * tr_state) f16     (PSUM read, scale at read)
  DVE : trep = max(min(u, 1), -1)           (one fp16 tensor_scalar)
  SP  : weight streams + per-layer out DMA (HW DGE)
"""
import sys
import numpy as np

sys.path.insert(0, "/opt/trn_rl_repo")

from contextlib import ExitStack

import concourse.bass as bass
import concourse.tile as tile
from concourse import mybir
from concourse.bass_utils import run_bass_kernel_spmd

B = 16384
K = 30
LAYERS = 90
VL = 60
ZL = 240
NCORES = 8
BC = B // NCORES          # 2048
NCHUNK = 4
CH = BC // NCHUNK         # 512

F32 = mybir.dt.float32
MM_DT = mybir.dt.float32r   # fp32 data on the fast PE path (1 cyc/row at N>=256)
F16 = mybir.dt.float16

AO = mybir.AluOpType
RELU = mybir.ActivationFunctionType.Relu
IDENT = mybir.ActivationFunctionType.Identity
LAST_RESULT = None  # BassKernelResults of the most recent run (for profiling)


def build_kernel(inv_kap):
    nc = bass.Bass()

    hr_in = nc.declare_dram_parameter("HrT", [K, BC], MM_DT, isOutput=False)
    hhy_in = nc.declare_dram_parameter("HHY", [120, 8 * BC], F16, isOutput=False)
    w1a_in = nc.declare_dram_parameter("W1A", [LAYERS, 126, ZL], MM_DT, isOutput=False)
    w1b_in = nc.declare_dram_parameter("W1B", [LAYERS, K, ZL], F16, isOutput=False)
    w2ra_in = nc.declare_dram_parameter("W2RA", [LAYERS, 128, 120], MM_DT, isOutput=False)
    w2rb_in = nc.declare_dram_parameter("W2RB", [LAYERS, 113, 120], MM_DT, isOutput=False)
    w3a_in = nc.declare_dram_parameter("W3A", [LAYERS, 128, VL], MM_DT, isOutput=False)
    w3b_in = nc.declare_dram_parameter("W3B", [LAYERS, 112, VL], MM_DT, isOutput=False)
    bias_in = nc.declare_dram_parameter("BIAS", [LAYERS, 128, 3], F32, isOutput=False)
    sel_in = nc.declare_dram_parameter("SEL", [120, 8 * K], F16, isOutput=False)
    zero32_in = nc.declare_dram_parameter("ZERO32", [126, BC], MM_DT, isOutput=False)
    zero16_in = nc.declare_dram_parameter("ZERO16", [120, BC], F16, isOutput=False)
    ones_in = nc.declare_dram_parameter("ONES", [1, BC], MM_DT, isOutput=False)
    out_dram = nc.declare_dram_parameter("OUT", [LAYERS, K, BC], F16, isOutput=True)

    with tile.TileContext(nc) as tc, ExitStack() as ctx:
        persist = ctx.enter_context(tc.tile_pool(name="persist", bufs=1))
        wpool = ctx.enter_context(tc.tile_pool(name="w", bufs=3))
        zpool = ctx.enter_context(tc.tile_pool(name="z", bufs=3))
        upool = ctx.enter_context(tc.tile_pool(name="u", bufs=3))
        pp_st = ctx.enter_context(tc.tile_pool(name="ps_st", bufs=1, space="PSUM"))
        pp_z = ctx.enter_context(tc.tile_pool(name="ps_z", bufs=1, space="PSUM"))
        pp_v = ctx.enter_context(tc.tile_pool(name="ps_v", bufs=1, space="PSUM"))
        pp_h = ctx.enter_context(tc.tile_pool(name="ps_h", bufs=1, space="PSUM"))

        # ---- persistent state
        cA = persist.tile([126, BC], MM_DT)     # v | tH | Hr
        trep = persist.tile([120, BC], F16)     # t rep4 (einsum + mm1 rhs + output)
        hhy = persist.tile([120, 8, BC], F16)   # HH rearranged
        pbuf = persist.tile([120, 8, BC], F16)  # einsum products
        sel = persist.tile([120, 8, K], F16)    # selector weights

        # t_tilde accumulators: one persistent PSUM bank per chunk
        trst = [
            pp_st.tile([120, CH], F32, tag=f"trst{c}", name=f"trst{c}")
            for c in range(NCHUNK)
        ]

        nc.gpsimd.dma_start(cA[:], zero32_in[:])
        nc.gpsimd.dma_start(trep[:], zero16_in[:])
        nc.gpsimd.dma_start(cA[96 : 96 + K, :], hr_in[:])
        nc.gpsimd.dma_start(hhy[:].rearrange("p a b -> p (a b)"), hhy_in[:])
        nc.gpsimd.dma_start(sel[:].rearrange("p a b -> p (a b)"), sel_in[:])

        for l in range(LAYERS):
            w1a = wpool.tile([126, ZL], MM_DT, tag="w1a")
            w1b = wpool.tile([K, ZL], F16, tag="w1b")
            w2ra = wpool.tile([128, 120], MM_DT, tag="w2ra")
            w2rb = wpool.tile([113, 120], MM_DT, tag="w2rb")
            w3a = wpool.tile([128, VL], MM_DT, tag="w3a")
            w3b = wpool.tile([112, VL], MM_DT, tag="w3b")
            bt = wpool.tile([128, 3], F32, tag="bias")
            nc.sync.dma_start(w1a[:], w1a_in[l])
            nc.sync.dma_start(w1b[:], w1b_in[l])
            nc.sync.dma_start(w2ra[:], w2ra_in[l])
            nc.sync.dma_start(w2rb[:], w2rb_in[l])
            nc.sync.dma_start(w3a[:], w3a_in[l])
            nc.sync.dma_start(w3b[:], w3b_in[l])
            nc.sync.dma_start(bt[:], bias_in[l])
            b1a = bt[0:128, 0:1]
            b1b = bt[0:112, 1:2]
            b3 = bt[0:VL, 2:3]

            ik = float(inv_kap[l])

            for c in range(NCHUNK):
                cs = bass.ts(c, CH)
                # ---- einsum producing tH for THIS layer (needs t of prev layer)
                if l > 0:
                    nc.vector.tensor_tensor(
                        pbuf[:, :, cs],
                        trep[:, cs].unsqueeze(1).broadcast_to((120, 8, CH)),
                        hhy[:, :, cs],
                        op=AO.mult,
                    )
                    thp = pp_h.tile([128, CH], F32, tag="th")
                    for g in range(8):
                        nc.tensor.matmul(
                            thp[64 : 64 + K, :],
                            sel[:, g, :],
                            pbuf[:, g, cs],
                            start=(g == 0),
                            stop=(g == 7),
                            tile_position=(0, 64),
                        )
                    nc.scalar.copy(cA[64 : 64 + K, cs], thp[64 : 64 + K, :])

                # ---- mm1: z pre-activation
                zp1 = pp_z.tile([128, CH], F32, tag="z1")
                zp2 = pp_z.tile([113, CH], F32, tag="z2")
                rA = cA[:, cs]
                rB = trep[0:K, cs]
                nc.tensor.matmul(zp1[:], w1a[:, 0:128], rA, start=True, stop=False)
                nc.tensor.matmul(zp1[:], w1b[:, 0:128], rB, start=False, stop=True)
                nc.tensor.matmul(zp2[0:112, :], w1a[:, 128:240], rA, start=True, stop=False)
                nc.tensor.matmul(zp2[0:112, :], w1b[:, 128:240], rB, start=False, stop=True)

                # ---- z = relu(zpre + b1) on ACT; zB row 112 is the b2 ones-row
                zA = zpool.tile([128, CH], MM_DT, tag="zA")
                zB = zpool.tile([113, CH], MM_DT, tag="zB")
                if l == 0:
                    nc.sync.dma_start(zB[112:113, :], ones_in[0:1, cs])
                nc.scalar.activation(zA[:], zp1[:], RELU, bias=b1a)
                nc.scalar.activation(zB[0:112, :], zp2[0:112, :], RELU, bias=b1b)

                # ---- mm3: dv ; mm2rep4: t_tilde accumulates in PSUM across layers
                vp = pp_v.tile([VL, CH], F32, tag="vp")
                nc.tensor.matmul(vp[:], w3a[:], zA[:], start=True, stop=False)
                nc.tensor.matmul(vp[:], w3b[:], zB[0:112, :], start=False, stop=True)
                tr = trst[c]
                nc.tensor.matmul(tr[:], w2ra[:], zA[:],
                                 start=(l == 0), stop=False)
                nc.tensor.matmul(tr[:], w2rb[:], zB[:],
                                 start=False, stop=(l == LAYERS - 1))

                # ---- v += vp + b3 (ACT to SBUF, then gpsimd accumulate-DMA)
                vtmp = zpool.tile([VL, CH], MM_DT, tag="vtmp")
                nc.scalar.activation(vtmp[:], vp[:], IDENT, bias=b3)
                nc.gpsimd.dma_start(cA[0:VL, cs], vtmp[:], accum_op=AO.add)

                # ---- t = clip(ik * t_tilde, -1, 1); scale applied at PSUM read
                u = upool.tile([120, CH], F16, tag="u")
                nc.scalar.activation(u[:], tr[:], IDENT, scale=ik)
                nc.vector.tensor_scalar(
                    trep[:, cs], u[:], 1.0, -1.0, op0=AO.min, op1=AO.max)

            # ---- emit t of this layer (block j4=0 of trep is plain t)
            nc.sync.dma_start(out_dram[l], trep[0:K, :])

    _split_waits(nc)
    return nc


def _split_waits(nc, limit=1):
    """This toolchain build only accepts one sem-wait per instruction;
    hoist surplus waits onto same-engine NoOps inserted before the inst."""
    ctr = 0
    for f in nc.m.functions:
        for blk in f.blocks:
            insts = blk.instructions
            if not any(
                i.sync_info and i.sync_info.on_wait and len(i.sync_info.on_wait) > limit
                for i in insts
            ):
                continue
            new = []
            for inst in insts:
                si = inst.sync_info
                if si and si.on_wait and len(si.on_wait) > limit:
                    waits = list(si.on_wait)
                    extra, keep = waits[:-limit], waits[-limit:]
                    for w in extra:
                        ctr += 1
                        n = mybir.InstNoOp(name=f"WSPLIT-{ctr}", ins=[], outs=[])
                        n.engine = inst.engine
                        n.sync_info = mybir.SyncInfo(on_wait=[w], on_update=[])
                        new.append(n)
                    si.on_wait = keep
                new.append(inst)
            blk.instructions = new
    return ctr


def _prep_shared(W1, b1, W2, b2, W3, b3):
    L = W1.shape[0]
    # cA row order: [v(0:60) | pad4 | tH(64:94) | pad2 | Hr(96:126)]
    W1A = np.zeros((L, 126, ZL), np.float32)
    W1A[:, 0:VL] = W1[:, :, 30:90].transpose(0, 2, 1)        # v cols
    W1A[:, 64:94] = W1[:, :, 120:150].transpose(0, 2, 1)     # tH cols
    W1A[:, 96:126] = W1[:, :, 0:30].transpose(0, 2, 1)       # Hr cols
    W1B = np.ascontiguousarray(
        W1[:, :, 90:120].transpose(0, 2, 1)).astype(np.float16)  # t cols [L,30,240]

    # rep4 W2: out partition p=(j4*30+k) gets W2 row k; b2 rides the ones-row
    W2T = W2.transpose(0, 2, 1)                              # [L, 240, 30]
    W2R = np.tile(W2T, (1, 1, 4))                            # [L, 240, 120] col p -> p%30
    W2RA = np.ascontiguousarray(W2R[:, 0:128])
    W2RB = np.zeros((L, 113, 120), np.float32)
    W2RB[:, 0:112] = W2R[:, 128:240]
    W2RB[:, 112] = np.tile(b2, (1, 4))                       # ones-row bias

    W3T = W3.transpose(0, 2, 1)
    W3A = np.ascontiguousarray(W3T[:, 0:128])
    W3B = np.ascontiguousarray(W3T[:, 128:240])

    BIAS = np.zeros((L, 128, 3), np.float32)
    BIAS[:, 0:128, 0] = b1[:, 0:128]
    BIAS[:, 0:112, 1] = b1[:, 128:240]
    BIAS[:, 0:VL, 2] = b3

    # selector: thp[j] = sum_k t[k]*HH[k,j];  p = j4*30+k, group g = j//4
    SEL = np.zeros((120, 8, K), np.float16)
    for j in range(K):
        g, j4 = j // 4, j % 4
        SEL[j4 * K : (j4 + 1) * K, g, j] = 1.0
    return W1A, W1B, W2RA, W2RB, W3A, W3B, BIAS, SEL.reshape(120, 8 * K)


def kernel(Hr, HH, W1, b1, W2, b2, W3, b3, kappa):
    Hr = np.asarray(Hr, np.float32)
    HH = np.asarray(HH, np.float32)
    W1 = np.asarray(W1, np.float32)
    b1 = np.asarray(b1, np.float32)
    W2 = np.asarray(W2, np.float32)
    b2 = np.asarray(b2, np.float32)
    W3 = np.asarray(W3, np.float32)
    b3 = np.asarray(b3, np.float32)
    kappa = np.asarray(kappa, np.float32)

    W1A, W1B, W2RA, W2RB, W3A, W3B, BIAS, SELh = _prep_shared(W1, b1, W2, b2, W3, b3)
    inv_kap = (1.0 / np.abs(kappa)).astype(np.float32)

    in_maps = []
    for ci in range(NCORES):
        sl = slice(ci * BC, (ci + 1) * BC)
        HrT = np.ascontiguousarray(Hr[sl].T)
        HHp = np.zeros((BC, K, 32), np.float32)
        HHp[:, :, :K] = HH[sl]
        # HHY[p=(j4*30+k), (g, b)] = HH[b, k, g*4+j4]
        HHY = HHp.reshape(BC, K, 8, 4).transpose(3, 1, 2, 0).reshape(120, 8 * BC)
        in_maps.append({
            "HrT": HrT, "HHY": HHY.astype(np.float16),
            "W1A": W1A, "W1B": W1B, "W2RA": W2RA, "W2RB": W2RB,
            "W3A": W3A, "W3B": W3B,
            "BIAS": BIAS, "SEL": SELh,
            "ZERO32": np.zeros((126, BC), np.float32),
            "ZERO16": np.zeros((120, BC), np.float16),
            "ONES": np.ones((1, BC), np.float32),
        })

    nc = build_kernel(inv_kap)
    res = run_bass_kernel_spmd(nc, in_maps, list(range(NCORES)))
    global LAST_RESULT
    LAST_RESULT = res
    out = np.concatenate(
        [r["OUT"].transpose(0, 2, 1) for r in res.results], axis=1
    )
    return np.ascontiguousarray(out.astype(np.float32))


# revision 19
# speedup vs baseline: 2.4125x; 1.2247x over previous
"""DetNet Trainium2 kernel v7: 90-layer MLP recurrence, data-parallel over 8 cores.

Per core (2048 samples), features on partitions, batch on free axis.
Partition layouts keep every engine op same-base (non-0 bases 32-aligned,
<=32 rows):
  cA   [126,2048] f32r: v(0:60) | pad | tH(64:94) | pad | Hr(96:126)
  trep [120,2048] f16 : t replicated 4x (block j4 at partitions 30*j4+k);
       produced rep4 for free by the W2-rep4 matmul (M is free on PE)
  ttr  [120,2048] f32 : t_tilde state, rep4

Per layer, per chunk (CH=512):
  DVE : P[(j4,k),(g,b)] = trep * hhy        (all-fp16, 2x mode)
  PE  : 8 selector matmuls (fp16) -> pvh[64:94] psum
  ACT : copy pvh[64:94] -> cA[64:94]
  PE  : mm1 z = W1 @ [cA; t]   (cA k-tile fp32r, t k-tile fp16)
  ACT : zA=relu(zp1+b1a) f16, zB=relu(zp2+b1b) f16
  PE  : mm3 pvh[0:60] = W3f16 @ z ; mm2r tr = W2rep4f16 @ z  (all fp16)
  ACT : vtmp = Identity(pvh[0:60] + b3)
  Pool: cA[0:60] += vtmp  (gpsimd DMA accum_op=add)
  DVE : ttr += tr + b2rep  (scalar_tensor_tensor)
  ACT : u = Identity(ik * ttr) f16
  DVE : trep = max(min(u, 1), -1)  (fp16 fast mode)
  SP  : weight streams + per-layer out DMA (HW DGE)
PSUM: zp1, zp2, tr, pvh(vp 0:60 | thp 64:94) all double-buffered = 8 banks.
"""
import sys
import numpy as np

sys.path.insert(0, "/opt/trn_rl_repo")

from contextlib import ExitStack

import concourse.bass as bass
import concourse.tile as tile
from concourse import mybir
from concourse.bass_utils import run_bass_kernel_spmd

B = 16384
K = 30
LAYERS = 90
VL = 60
ZL = 240
NCORES = 8
BC = B // NCORES          # 2048
NCHUNK = 4
CH = BC // NCHUNK         # 512

F32 = mybir.dt.float32
MM_DT = mybir.dt.float32r   # fp32 data on the fast PE path (1 cyc/row at N>=256)
F16 = mybir.dt.float16

AO = mybir.AluOpType
RELU = mybir.ActivationFunctionType.Relu
IDENT = mybir.ActivationFunctionType.Identity
LAST_RESULT = None  # BassKernelResults of the most recent run (for profiling)


def build_kernel(inv_kap):
    nc = bass.Bass()

    hr_in = nc.declare_dram_parameter("HrT", [K, BC], MM_DT, isOutput=False)
    hhy_in = nc.declare_dram_parameter("HHY", [120, 8 * BC], F16, isOutput=False)
    w1a_in = nc.declare_dram_parameter("W1A", [LAYERS, 126, ZL], MM_DT, isOutput=False)
    w1b_in = nc.declare_dram_parameter("W1B", [LAYERS, K, ZL], F16, isOutput=False)
    w2ra_in = nc.declare_dram_parameter("W2RA", [LAYERS, 128, 120], F16, isOutput=False)
    w2rb_in = nc.declare_dram_parameter("W2RB", [LAYERS, 112, 120], F16, isOutput=False)
    w3a_in = nc.declare_dram_parameter("W3A", [LAYERS, 128, VL], F16, isOutput=False)
    w3b_in = nc.declare_dram_parameter("W3B", [LAYERS, 112, VL], F16, isOutput=False)
    bias_in = nc.declare_dram_parameter("BIAS", [LAYERS, 128, 4], F32, isOutput=False)
    sel_in = nc.declare_dram_parameter("SEL", [120, 8 * K], F16, isOutput=False)
    zero32_in = nc.declare_dram_parameter("ZERO32", [126, BC], MM_DT, isOutput=False)
    zero16_in = nc.declare_dram_parameter("ZERO16", [120, BC], F16, isOutput=False)
    out_dram = nc.declare_dram_parameter("OUT", [LAYERS, K, BC], F16, isOutput=True)

    with tile.TileContext(nc) as tc, ExitStack() as ctx:
        persist = ctx.enter_context(tc.tile_pool(name="persist", bufs=1))
        wpool = ctx.enter_context(tc.tile_pool(name="w", bufs=3))
        zpool = ctx.enter_context(tc.tile_pool(name="z", bufs=3))
        upool = ctx.enter_context(tc.tile_pool(name="u", bufs=3))
        pp_z = ctx.enter_context(tc.tile_pool(name="ps_z", bufs=2, space="PSUM"))
        pp_t = ctx.enter_context(tc.tile_pool(name="ps_t", bufs=2, space="PSUM"))
        pp_vh = ctx.enter_context(tc.tile_pool(name="ps_vh", bufs=2, space="PSUM"))

        # ---- persistent state
        cA = persist.tile([126, BC], MM_DT)     # v | tH | Hr
        trep = persist.tile([120, BC], F16)     # t rep4 (einsum + mm1 rhs + output)
        ttr = persist.tile([120, BC], F32)      # t_tilde rep4
        hhy = persist.tile([120, 8, BC], F16)   # HH rearranged
        pbuf = persist.tile([120, 8, BC], F16)  # einsum products
        sel = persist.tile([120, 8, K], F16)    # selector weights

        nc.gpsimd.dma_start(cA[:], zero32_in[:])
        nc.gpsimd.dma_start(trep[:], zero16_in[:])
        nc.vector.memset(ttr[:], 0.0)
        nc.gpsimd.dma_start(cA[96 : 96 + K, :], hr_in[:])
        nc.gpsimd.dma_start(hhy[:].rearrange("p a b -> p (a b)"), hhy_in[:])
        nc.gpsimd.dma_start(sel[:].rearrange("p a b -> p (a b)"), sel_in[:])

        for l in range(LAYERS):
            w1a = wpool.tile([126, ZL], MM_DT, tag="w1a")
            w1b = wpool.tile([K, ZL], F16, tag="w1b")
            w2ra = wpool.tile([128, 120], F16, tag="w2ra")
            w2rb = wpool.tile([112, 120], F16, tag="w2rb")
            w3a = wpool.tile([128, VL], F16, tag="w3a")
            w3b = wpool.tile([112, VL], F16, tag="w3b")
            bt = wpool.tile([128, 4], F32, tag="bias")
            nc.sync.dma_start(w1a[:], w1a_in[l])
            nc.sync.dma_start(w1b[:], w1b_in[l])
            nc.sync.dma_start(w2ra[:], w2ra_in[l])
            nc.sync.dma_start(w2rb[:], w2rb_in[l])
            nc.sync.dma_start(w3a[:], w3a_in[l])
            nc.sync.dma_start(w3b[:], w3b_in[l])
            nc.sync.dma_start(bt[:], bias_in[l])
            b1a = bt[0:128, 0:1]
            b1b = bt[0:112, 1:2]
            b3 = bt[0:VL, 2:3]
            b2r = bt[0:120, 3:4]

            ik = float(inv_kap[l])

            for c in range(NCHUNK):
                cs = bass.ts(c, CH)
                pvh = pp_vh.tile([128, CH], F32, tag="pvh")  # vp 0:60 | thp 64:94
                # ---- einsum producing tH for THIS layer (needs t of prev layer)
                if l > 0:
                    nc.vector.tensor_tensor(
                        pbuf[:, :, cs],
                        trep[:, cs].unsqueeze(1).broadcast_to((120, 8, CH)),
                        hhy[:, :, cs],
                        op=AO.mult,
                    )
                    for g in range(8):
                        nc.tensor.matmul(
                            pvh[64 : 64 + K, :],
                            sel[:, g, :],
                            pbuf[:, g, cs],
                            start=(g == 0),
                            stop=(g == 7),
                            tile_position=(0, 64),
                        )
                    nc.scalar.copy(cA[64 : 64 + K, cs], pvh[64 : 64 + K, :])

                # ---- mm1: z pre-activation
                zp1 = pp_z.tile([128, CH], F32, tag="z1")
                zp2 = pp_z.tile([112, CH], F32, tag="z2")
                rA = cA[:, cs]
                rB = trep[0:K, cs]
                nc.tensor.matmul(zp1[:], w1a[:, 0:128], rA, start=True, stop=False)
                nc.tensor.matmul(zp1[:], w1b[:, 0:128], rB, start=False, stop=True)
                nc.tensor.matmul(zp2[:], w1a[:, 128:240], rA, start=True, stop=False)
                nc.tensor.matmul(zp2[:], w1b[:, 128:240], rB, start=False, stop=True)

                # ---- z = relu(zpre + b1) on ACT, fp16 out
                zA = zpool.tile([128, CH], F16, tag="zA")
                zB = zpool.tile([112, CH], F16, tag="zB")
                nc.scalar.activation(zA[:], zp1[:], RELU, bias=b1a)
                nc.scalar.activation(zB[:], zp2[:], RELU, bias=b1b)

                # ---- mm3: dv into pvh[0:60]; mm2rep4: dt into tr (all fp16)
                nc.tensor.matmul(pvh[0:VL, :], w3a[:], zA[:], start=True, stop=False)
                nc.tensor.matmul(pvh[0:VL, :], w3b[:], zB[:], start=False, stop=True)
                tr = pp_t.tile([120, CH], F32, tag="tr")
                nc.tensor.matmul(tr[:], w2ra[:], zA[:], start=True, stop=False)
                nc.tensor.matmul(tr[:], w2rb[:], zB[:], start=False, stop=True)

                # ---- v += vp + b3 (ACT to SBUF, then gpsimd accumulate-DMA)
                vtmp = zpool.tile([VL, CH], MM_DT, tag="vtmp")
                nc.scalar.activation(vtmp[:], pvh[0:VL, :], IDENT, bias=b3)
                nc.gpsimd.dma_start(cA[0:VL, cs], vtmp[:], accum_op=AO.add)

                # ---- t_tilde += dt + b2 (DVE, PSUM read)
                nc.vector.scalar_tensor_tensor(
                    ttr[:, cs], tr[:], b2r, ttr[:, cs], op0=AO.add, op1=AO.add)

                # ---- t = clip(ik * t_tilde, -1, 1)
                u = upool.tile([120, CH], F16, tag="u")
                nc.scalar.activation(u[:], ttr[:, cs], IDENT, scale=ik)
                nc.vector.tensor_scalar(
                    trep[:, cs], u[:], 1.0, -1.0, op0=AO.min, op1=AO.max)

            # ---- emit t of this layer (block j4=0 of trep is plain t)
            nc.sync.dma_start(out_dram[l], trep[0:K, :])

    _split_waits(nc)
    return nc


def _split_waits(nc, limit=1):
    """This toolchain build only accepts one sem-wait per instruction;
    hoist surplus waits onto same-engine NoOps inserted before the inst."""
    ctr = 0
    for f in nc.m.functions:
        for blk in f.blocks:
            insts = blk.instructions
            if not any(
                i.sync_info and i.sync_info.on_wait and len(i.sync_info.on_wait) > limit
                for i in insts
            ):
                continue
            new = []
            for inst in insts:
                si = inst.sync_info
                if si and si.on_wait and len(si.on_wait) > limit:
                    waits = list(si.on_wait)
                    extra, keep = waits[:-limit], waits[-limit:]
                    for w in extra:
                        ctr += 1
                        n = mybir.InstNoOp(name=f"WSPLIT-{ctr}", ins=[], outs=[])
                        n.engine = inst.engine
                        n.sync_info = mybir.SyncInfo(on_wait=[w], on_update=[])
                        new.append(n)
                    si.on_wait = keep
                new.append(inst)
            blk.instructions = new
    return ctr


def _prep_shared(W1, b1, W2, b2, W3, b3):
    L = W1.shape[0]
    # cA row order: [v(0:60) | pad4 | tH(64:94) | pad2 | Hr(96:126)]
    W1A = np.zeros((L, 126, ZL), np.float32)
    W1A[:, 0:VL] = W1[:, :, 30:90].transpose(0, 2, 1)        # v cols
    W1A[:, 64:94] = W1[:, :, 120:150].transpose(0, 2, 1)     # tH cols
    W1A[:, 96:126] = W1[:, :, 0:30].transpose(0, 2, 1)       # Hr cols
    W1B = np.ascontiguousarray(
        W1[:, :, 90:120].transpose(0, 2, 1)).astype(np.float16)  # t cols [L,30,240]

    # rep4 W2: out partition p=(j4*30+k) gets W2 row p%30
    W2T = W2.transpose(0, 2, 1)                              # [L, 240, 30]
    W2R = np.tile(W2T, (1, 1, 4)).astype(np.float16)
    W2RA = np.ascontiguousarray(W2R[:, 0:128])
    W2RB = np.ascontiguousarray(W2R[:, 128:240])
    W3T = W3.transpose(0, 2, 1).astype(np.float16)
    W3A = np.ascontiguousarray(W3T[:, 0:128])
    W3B = np.ascontiguousarray(W3T[:, 128:240])

    BIAS = np.zeros((L, 128, 4), np.float32)
    BIAS[:, 0:128, 0] = b1[:, 0:128]
    BIAS[:, 0:112, 1] = b1[:, 128:240]
    BIAS[:, 0:VL, 2] = b3
    BIAS[:, 0:120, 3] = np.tile(b2, (1, 4))

    # selector: thp[j] = sum_k t[k]*HH[k,j];  p = j4*30+k, group g = j//4
    SEL = np.zeros((120, 8, K), np.float16)
    for j in range(K):
        g, j4 = j // 4, j % 4
        SEL[j4 * K : (j4 + 1) * K, g, j] = 1.0
    return W1A, W1B, W2RA, W2RB, W3A, W3B, BIAS, SEL.reshape(120, 8 * K)


def kernel(Hr, HH, W1, b1, W2, b2, W3, b3, kappa):
    Hr = np.asarray(Hr, np.float32)
    HH = np.asarray(HH, np.float32)
    W1 = np.asarray(W1, np.float32)
    b1 = np.asarray(b1, np.float32)
    W2 = np.asarray(W2, np.float32)
    b2 = np.asarray(b2, np.float32)
    W3 = np.asarray(W3, np.float32)
    b3 = np.asarray(b3, np.float32)
    kappa = np.asarray(kappa, np.float32)

    W1A, W1B, W2RA, W2RB, W3A, W3B, BIAS, SELh = _prep_shared(W1, b1, W2, b2, W3, b3)
    inv_kap = (1.0 / np.abs(kappa)).astype(np.float32)

    in_maps = []
    for ci in range(NCORES):
        sl = slice(ci * BC, (ci + 1) * BC)
        HrT = np.ascontiguousarray(Hr[sl].T)
        HHp = np.zeros((BC, K, 32), np.float32)
        HHp[:, :, :K] = HH[sl]
        # HHY[p=(j4*30+k), (g, b)] = HH[b, k, g*4+j4]
        HHY = HHp.reshape(BC, K, 8, 4).transpose(3, 1, 2, 0).reshape(120, 8 * BC)
        in_maps.append({
            "HrT": HrT, "HHY": HHY.astype(np.float16),
            "W1A": W1A, "W1B": W1B, "W2RA": W2RA, "W2RB": W2RB,
            "W3A": W3A, "W3B": W3B,
            "BIAS": BIAS, "SEL": SELh,
            "ZERO32": np.zeros((126, BC), np.float32),
            "ZERO16": np.zeros((120, BC), np.float16),
        })

    nc = build_kernel(inv_kap)
    res = run_bass_kernel_spmd(nc, in_maps, list(range(NCORES)))
    global LAST_RESULT
    LAST_RESULT = res
    out = np.concatenate(
        [r["OUT"].transpose(0, 2, 1) for r in res.results], axis=1
    )
    return np.ascontiguousarray(out.astype(np.float32))
